# revision 74
# baseline (speedup 1.0000x reference)
"""CARAFE content-aware upsampling on 8 Trainium2 NeuronCores (Bass/Tile).

Problem: x[2,256,64,64], 1x1 compress conv (256->32), 5x5 encoder conv
(32->100), pixel-shuffle(r=2) + softmax over 25 taps, then dynamic-filter
reassembly: out[b,c,2h+r1,2w+r2] = sum_k x[b,c,h+di,w+dj] * softmax_w.

Sharding: pure data-parallel over (batch, 16-row H slices) -> 8 cores.

Per-core pipeline (DMA-dispatch-minimal redesign, all-bf16 matmuls at
1 cyc/row; rel-tol is 2e-2 so bf16 error ~0.5% is fine):
  - compress (1x1) on PE; a 4-way shifted copy of y1 on 128 partitions
    (y1stack) lets the encoder contract 2x2 tap blocks, so the 5x5 conv
    is 11 matmuls per row-parity instead of 25.
  - softmax stays channel-major; a [100,128] permutation matmul then
    regroups (25 taps x 4 subpixels) partitions to taps-only partitions
    with sub on the free axis (32-aligned PSUM slices), after a
    broadcast matmul + one DVE multiply fold in the 1/sum.
  - the banded 25-tap MAC operand is built via a DRAM round trip: the
    diagonal (partition+free coupled) strides live entirely on the DRAM
    side, so the whole scatter is 10 large DMAs into a pre-zeroed DRAM
    scratch (ExternalOutput buffers arrive zeroed = the band gaps) + 6
    row-chunk loads back, instead of 160 per-column SBUF scatters.
  - x windows arrive pre-transposed from the host (xt2, bf16) and are
    gathered into the [120, g*b4*c] xcall operand by 6 SWDGE DMAs.
  - MAC: 32 matmuls (stationary = band block, moving = xcall 256-chan
    slice), psum [128n, 256c] pairs, bf16 results upcast on the host.
"""

import sys

sys.path.insert(0, "/opt/trn_rl_repo")

import numpy as np

import concourse.bacc as bacc
import concourse.bass as bass
import concourse.tile as tile
from concourse import mybir
from concourse.ap import AP

F32 = mybir.dt.float32
F32R = mybir.dt.float32r
BF16 = mybir.dt.bfloat16

# geometry
B, C, H, W = 2, 256, 64, 64
RATIO, K_UP, C_MID, ENC_K = 2, 5, 32, 5
NK = RATIO * RATIO * K_UP * K_UP  # 100
HSLICE = 16                       # output source rows per core
ROWS = HSLICE + 4                 # with 2-row halo each side
WP = W + 4                        # padded width
PADPOS = ROWS * WP                # 1360
NCORES = 8

KDIM = 120                        # 6 rows x 20 cols window pixels
YF = 4096                         # ybig free dim: col = 32*n + blk
XCF = 8192                        # xcall free dim: (g, b4, c)
YMSF = 2048                       # yMs free dim: (w, sub, tb)
WPKW = 296                        # wpk cols: wcat 64 | sel 4 | Eall 128 | selt 100


def build_program(with_ebias: bool):
    nc = bacc.Bacc()
    xs_d = nc.declare_dram_parameter("xs", [2, 128, PADPOS], BF16, isOutput=False)
    xt2_d = nc.declare_dram_parameter("xt2", [ROWS, 20, 4, C], BF16, isOutput=False)
    wpk_d = nc.declare_dram_parameter("wpk", [128, WPKW], BF16, isOutput=False)
    wet_d = nc.declare_dram_parameter("wet32", [C_MID, 25 * NK], BF16, isOutput=False)
    wetq_d = nc.declare_dram_parameter("wetq", [128, 600], BF16, isOutput=False)
    if with_ebias:
        ebias_d = nc.declare_dram_parameter("ebias", [2, NK, 512], F32, isOutput=False)
    # band scratch; ExternalOutput buffers are pre-zeroed by the runtime,
    # which supplies the structural zeros between the band diagonals.
    ydram_d = nc.declare_dram_parameter("ydram", [KDIM, YF], BF16, isOutput=True)
    out_d = nc.declare_dram_parameter("out", [128, 8192], BF16, isOutput=True)

    with tile.TileContext(nc) as tc:
        # The diagonal DRAM scatter APs (partition+free coupled strides on
        # the DRAM side) confuse the byte-range race detector; dependency
        # generation itself is tensor-granular and conservative.
        tc.race_detector_enabled = False
        with tc.tile_pool(name="persist", bufs=1) as pp:
            # ---- input loads (split so compress can start early) ----
            wpk = pp.tile([128, WPKW], BF16, tag="wpk")
            nc.scalar.dma_start(wpk[:], wpk_d[:])

            xst = []
            for ct in range(2):
                t = pp.tile([128, PADPOS], BF16, name=f"xst{ct}", tag=f"xs{ct}")
                xst.append(t)
            for piece in ((0, 512), (512, 1024), (1024, PADPOS)):
                for ct in range(2):
                    eng = nc.sync if ct == 0 else nc.scalar
                    eng.dma_start(
                        xst[ct][:, piece[0]:piece[1]],
                        xs_d[ct, :, piece[0]:piece[1]],
                    )

            wetb = pp.tile([C_MID, 25 * NK], BF16, tag="wetb")
            nc.scalar.dma_start(wetb[:], wet_d[:])
            wetq = pp.tile([128, 600], BF16, tag="wetq")
            nc.scalar.dma_start(wetq[:], wetq_d[:])

            if with_ebias:
                ebias = []
                for ro in range(2):
                    t = pp.tile([NK, 512], F32, tag=f"ebias{ro}")
                    nc.scalar.dma_start(t[:], ebias_d[ro])
                    ebias.append(t)

            # xcall[rq*20+cq, (g, b4, c)] = x[c, 2g+rq, b4*16+cq] (padded
            # coords), gathered from the host-transposed xt2 on the SWDGE
            # queue (Pool engine) to keep HWDGE free.
            xcall = pp.tile([KDIM, XCF], BF16, tag="xcall")
            for rq in range(6):
                dst = AP(xcall.tensor, rq * 20 * XCF,
                         [[XCF, 20], [1024, 8], [1, 1024]])
                src = AP(xt2_d, rq * 20480,
                         [[1024, 20], [40960, 8], [1, 1024]])
                nc.gpsimd.dma_start(dst, src)

            # ---- compress conv y1[32, PADPOS] ----
            psCMP = tc.alloc_tile_pool(name="psCMP", bufs=2, space="PSUM")
            y1 = pp.tile([C_MID, PADPOS], BF16, tag="y1")
            # shifted stack alongside y1: y1stack[tl*32+m, p] = y1[m, p+s_tl]
            # with s_tl in {0, WP, 1, WP+1}, so a 2x2 tap block contracts
            # 128 partitions. Built per compress chunk straight from PSUM.
            STK = PADPOS - WP - 1
            SHIFTS = (0, WP, 1, WP + 1)
            y1stack = pp.tile([128, PADPOS], BF16, tag="y1stack")
            off = 0
            ci = 0
            while off < PADPOS:
                n = min(512, PADPOS - off)
                ps = psCMP.tile([C_MID, 512], F32, tag="cmp")
                for ct in range(2):
                    nc.tensor.matmul(
                        ps[:, :n],
                        wpk[:, ct * 32:(ct + 1) * 32],
                        xst[ct][:, off:off + n],
                        start=(ct == 0), stop=(ct == 1),
                    )
                eng = nc.vector if ci % 2 == 0 else nc.scalar
                if eng is nc.vector:
                    eng.tensor_copy(y1[:, off:off + n], ps[:, :n])
                else:
                    eng.copy(y1[:, off:off + n], ps[:, :n])
                # stack slices fed by this chunk (read from y1 SBUF so the
                # psum tile is released immediately): stack cols
                # [off-s, off+n-s) <- y1 cols [off, off+n)
                for tl, sh in enumerate(SHIFTS):
                    a = max(0, off - sh)
                    b = min(STK, off + n - sh)
                    if b <= a:
                        continue
                    dst = y1stack[tl * 32:(tl + 1) * 32, a:b]
                    src = y1[:, a + sh:b + sh]
                    if tl % 3 == 0:
                        nc.gpsimd.tensor_copy(dst, src)
                    elif tl % 3 == 1:
                        nc.vector.tensor_copy(dst, src)
                    else:
                        nc.scalar.copy(dst, src)
                off += n
                ci += 1


            psCMP.release()
            psENC = tc.alloc_tile_pool(name="psENC", bufs=2, space="PSUM")
            psS = tc.alloc_tile_pool(name="psS", bufs=1, space="PSUM")
            psRG = tc.alloc_tile_pool(name="psRG", bufs=2, space="PSUM")
            psB = tc.alloc_tile_pool(name="psB", bufs=2, space="PSUM")

            # ---- encoder conv + exp, per output-row parity ro ----
            # rhs columns stream in pos' = (w, tile, b4) order:
            # f = 32*w + 4*tile + b4  <->  (h = 2*tile + ro, wcol = 16*b4 + w)
            # 11 matmuls: 5 singles (dii=4, from y1), 2 pairs (djj=4, from
            # the {0,WP} stack rows), 4 quads (2x2 tap blocks, full stack).
            def encode(ro):
                ps = psENC.tile([NK, 512], F32, tag="enc")
                POSDIMS = [[1, 16], [2 * WP, 8], [16, 4]]
                groups = []
                for djj in range(5):    # singles: tap (dii=4, djj)
                    groups.append((y1.tensor, C_MID,
                                   (ro + 4) * WP + djj,
                                   wetb[:, (20 + djj) * NK:(21 + djj) * NK]))
                for gi, dii0 in enumerate((0, 2)):   # pairs: (dii0+a, djj=4)
                    groups.append((y1stack.tensor, 64,
                                   (ro + dii0) * WP + 4,
                                   wetq[0:64, 400 + gi * 100:500 + gi * 100]))
                qi = 0
                for dii0 in (0, 2):                  # quads
                    for djj0 in (0, 2):
                        groups.append((y1stack.tensor, 128,
                                       (ro + dii0) * WP + djj0,
                                       wetq[:, qi * 100:(qi + 1) * 100]))
                        qi += 1
                for i, (mt, cp, moff, statw) in enumerate(groups):
                    rhs = AP(mt, moff, [[PADPOS, cp]] + POSDIMS)
                    nc.tensor.matmul(
                        ps[:], statw, rhs,
                        start=(i == 0), stop=(i == len(groups) - 1),
                    )
                y2e = pp.tile([NK, 512], BF16, tag=f"y2e{ro}")
                if with_ebias:
                    nc.vector.scalar_tensor_tensor(
                        y2e[:], ps[:], 1.0, ebias[ro][:],
                        op0=mybir.AluOpType.mult, op1=mybir.AluOpType.add,
                    )
                    nc.scalar.activation(
                        y2e[:], y2e[:], mybir.ActivationFunctionType.Exp
                    )
                else:
                    nc.scalar.activation(
                        y2e[:], ps[:], mybir.ActivationFunctionType.Exp
                    )
                return y2e

            # tap-sum via select matmul [100,4] -> [4, 512], then reciprocal
            def tapsums(y2e, ro):
                ps = psS.tile([4, 512], F32, tag="sums")
                nc.tensor.matmul(ps[:], wpk[0:NK, 64:68],
                                 y2e[:], start=True, stop=True)
                rs4 = pp.tile([4, 512], BF16, tag=f"rs4{ro}")
                with nc.allow_low_precision(
                    reason="softmax 1/sum in bf16; rel-tol is 2e-2"
                ):
                    nc.vector.reciprocal(rs4[:], ps[:])
                return rs4

            # normalize (broadcast 1/sum over partitions via selt matmul,
            # one DVE multiply), then regroup to taps-on-partitions with
            # sub on the free axis via 4 select matmuls + copies:
            #   yMs[kk, 128*w + 32*sub + tb] = y2e[4*kk+sub, f] * rs4[sub, f]
            def regroup(y2e, rs4, ro, yms):
                pB = psB.tile([NK, 512], F32, tag="bc")
                nc.tensor.matmul(
                    pB[:], wpk[0:4, 196:296], rs4[:], start=True, stop=True,
                )
                ymn = pp.tile([NK, 512], BF16, name=f"ymn{ro}", tag=f"ymn{ro}")
                nc.vector.tensor_tensor(
                    ymn[:], y2e[:], pB[:], op=mybir.AluOpType.mult,
                )
                # one permutation matmul [100,100], then 4 psum-slice copies
                pG = psRG.tile([128, 512], F32, tag="rg")
                nc.tensor.matmul(
                    pG[:], wpk[0:NK, 68:196], ymn[:], start=True, stop=True,
                )
                for sub in range(4):
                    dst = AP(yms.tensor, sub * 32,
                             [[YMSF, 25], [128, 16], [1, 32]])
                    src = AP(pG.tensor, sub * 32 * 512,
                             [[512, 25], [32, 16], [1, 32]])
                    if sub % 2 == 0:
                        nc.vector.tensor_copy(dst, src)
                    else:
                        nc.scalar.copy(dst, src)

            # band scatter: per (ro, dii) one DMA into the DRAM scratch;
            # the w-diagonal (dst partition q = (ro+dii)*20 + w + djj AND
            # dst col 128*(16*ro + w) + ...) is a plain stride 4224 in flat
            # DRAM. Structural zeros come from the pre-zeroed output buf.
            def scatter(yms, ro):
                # ro0 avoids the Act queue (the ro1 regroup copies are
                # dispatched behind it); ro1 may use Act and spreads over
                # 3 queues so its last pieces land sooner.
                engs = (nc.sync, nc.gpsimd, nc.sync, nc.gpsimd, nc.sync)
                for dii in range(5):
                    src = AP(yms.tensor, dii * 5 * YMSF,
                             [[YMSF, 5], [128, 16], [1, 128]])
                    dst = AP(ydram_d,
                             (ro + dii) * 20 * YF + ro * 2048,
                             [[YF, 5], [YF + 128, 16], [1, 128]])
                    engs[dii].dma_start(dst, src)

            yms0 = pp.tile([25, YMSF], BF16, tag="yms0")
            yms1 = pp.tile([25, YMSF], BF16, tag="yms1")

            y2e0 = encode(0)
            rs40 = tapsums(y2e0, 0)
            y2e1 = encode(1)          # PE busy while DVE does recip ro0
            regroup(y2e0, rs40, 0, yms0)
            scatter(yms0, 0)
            rs41 = tapsums(y2e1, 1)
            regroup(y2e1, rs41, 1, yms1)
            scatter(yms1, 1)

            for pool in (psB, psRG, psS, psENC):
                pool.release()
            psMAC = tc.alloc_tile_pool(name="psMAC", bufs=6, space="PSUM")

            # ---- band load back (row-chunked) + 25-tap MAC ----
            # psum [n, c]: stationary = ybig block (strided cols), moving =
            # xcall full-channel slice (256 rows/matmul, 1 mm per block).
            ybig = pp.tile([KDIM, YF], BF16, tag="ybig")
            for i in range(6):
                r0, r1 = i * 20, (i + 1) * 20
                eng = (nc.gpsimd, nc.sync)[i % 2]
                eng.dma_start(
                    AP(ybig.tensor, r0 * YF, [[YF, r1 - r0], [1, YF]]),
                    AP(ydram_d, r0 * YF, [[YF, r1 - r0], [1, YF]]),
                )

            osbs = [pp.tile([128, 2048], BF16, name=f"osb{i}", tag=f"osb{i}")
                    for i in range(4)]
            for k in range(16):             # 2 blocks per psum tile
                ps = psMAC.tile([128, 512], F32, tag="mac")
                for j in range(2):
                    blk = k * 2 + j
                    nc.tensor.matmul(
                        ps[:, j * 256:(j + 1) * 256],
                        AP(ybig.tensor, blk, [[YF, KDIM], [32, 128]]),
                        AP(xcall.tensor, blk * 256, [[XCF, KDIM], [1, 256]]),
                        start=True, stop=True,
                    )
                osb = osbs[k // 4]
                col = (k % 4) * 512
                if k % 2 == 0:
                    nc.vector.tensor_copy(osb[:, col:col + 512], ps[:])
                else:
                    nc.scalar.copy(osb[:, col:col + 512], ps[:])
                if k % 2 == 1:
                    # ship each half-quarter as soon as its copies land
                    # (SP/Pool queues so Act SEQ stays free for copies);
                    # the final piece is split across two queues.
                    q, h = k // 4, (k % 4) // 2
                    base = q * 2048 + h * 1024
                    if k < 15:
                        eng = nc.gpsimd if k % 4 == 1 else nc.sync
                        eng.dma_start(
                            out_d[:, base:base + 1024],
                            osbs[q][:, h * 1024:h * 1024 + 1024],
                        )
                    else:
                        nc.sync.dma_start(
                            out_d[:, base:base + 512],
                            osbs[q][:, h * 1024:h * 1024 + 512],
                        )
                        nc.scalar.dma_start(
                            out_d[:, base + 512:base + 1024],
                            osbs[q][:, h * 1024 + 512:h * 1024 + 1024],
                        )
            psMAC.release()
    nc.compile()
    return nc


_CACHE: dict[bool, object] = {}


def _get_program(with_ebias: bool):
    if with_ebias not in _CACHE:
        _CACHE[with_ebias] = build_program(with_ebias)
    return _CACHE[with_ebias]


def _prep_inputs(x, w_comp, b_comp, w_enc, b_enc):
    """Build the per-core numpy input dicts."""
    import ml_dtypes

    bf16 = ml_dtypes.bfloat16
    x = np.asarray(x, dtype=np.float32)
    w_comp = np.asarray(w_comp, dtype=np.float32)
    b_comp = np.asarray(b_comp, dtype=np.float32)
    w_enc = np.asarray(w_enc, dtype=np.float32)
    b_enc = np.asarray(b_enc, dtype=np.float32)

    # packed weights: wcat | sel | Esel | esub
    wpk = np.zeros((128, WPKW), dtype=np.float32)
    for ct in range(2):
        wpk[:, ct * 32:(ct + 1) * 32] = w_comp[:, ct * 128:(ct + 1) * 128].T
    p = np.arange(NK)
    wpk[p, 64 + p % 4] = 1.0                       # sel
    wpk[p, 68 + (p % 4) * 32 + p // 4] = 1.0       # Eall (32-aligned subs)
    wpk[p % 4, 196 + p] = 1.0                      # selt

    we = w_enc.reshape(NK, C_MID, 25)              # [o, m, tap]
    wet32 = np.ascontiguousarray(
        np.transpose(we, (1, 2, 0)).reshape(C_MID, 25 * NK)
    )

    # wetq: stacked-encoder stationaries. Stack row tl = b*2 + a carries
    # shift a*WP + b, so quad (dii0, djj0) row (tl, m) = w_enc tap
    # (dii0+a, djj0+b); pairs cover (dii0+a, djj=4) on rows tl in {0,1}.
    wetq = np.zeros((128, 600), dtype=np.float32)
    wet_om = we.reshape(NK, C_MID, 5, 5)           # [o, m, dii, djj]
    qi = 0
    for dii0 in (0, 2):
        for djj0 in (0, 2):
            for tl in range(4):
                a, b = tl % 2, tl // 2
                wetq[tl * 32:(tl + 1) * 32, qi * 100:(qi + 1) * 100] = \
                    wet_om[:, :, dii0 + a, djj0 + b].T
            qi += 1
    for gi, dii0 in enumerate((0, 2)):
        for a in range(2):
            wetq[a * 32:(a + 1) * 32, 400 + gi * 100:500 + gi * 100] = \
                wet_om[:, :, dii0 + a, 4].T

    with_ebias = bool(b_comp.any() or b_enc.any())

    in_maps = []
    for core in range(NCORES):
        b = core // 4
        h0 = (core % 4) * HSLICE
        xpad = np.zeros((C, ROWS, WP), dtype=np.float32)
        r_lo = max(0, h0 - 2)
        r_hi = min(H, h0 + HSLICE + 2)
        xpad[:, (r_lo - (h0 - 2)):(r_hi - (h0 - 2)), 2:2 + W] = x[b, :, r_lo:r_hi, :]

        xt2 = np.empty((ROWS, 20, 4, C), dtype=bf16)
        for b4 in range(4):
            xt2[:, :, b4, :] = xpad[:, :, b4 * 16:b4 * 16 + 20].transpose(1, 2, 0)

        m = {
            "xs": np.ascontiguousarray(xpad.reshape(2, 128, PADPOS)).astype(bf16),
            "xt2": xt2,
            "wpk": wpk.astype(bf16),
            "wet32": wet32.astype(bf16),
            "wetq": wetq.astype(bf16),
        }
        if with_ebias:
            # field[o, h, w] = b_enc[o] + sum over valid taps of w_enc.b_comp
            wb = np.einsum("omt,m->ot", we, b_comp).reshape(NK, 5, 5)
            field = np.zeros((NK, HSLICE, W), dtype=np.float32)
            for di in range(-2, 3):
                for dj in range(-2, 3):
                    hh = np.arange(h0, h0 + HSLICE)[:, None] + di
                    ww = np.arange(W)[None, :] + dj
                    valid = ((hh >= 0) & (hh < H) & (ww >= 0) & (ww < W))
                    field += (
                        wb[:, di + 2, dj + 2][:, None, None]
                        * valid[None].astype(np.float32)
                    )
            field += b_enc[:, None, None]
            f = field.reshape(NK, 8, 2, 4, 16)        # (o, tile, ro, b4, w)
            f = np.transpose(f, (2, 0, 4, 1, 3))      # (ro, o, w, tile, b4)
            m["ebias"] = np.ascontiguousarray(f.reshape(2, NK, 512))
        in_maps.append(m)
    return in_maps, with_ebias


TRACE = False
LAST_RESULT = None


def kernel(x, w_comp, b_comp, w_enc, b_enc):
    global LAST_RESULT
    from concourse.bass_utils import run_bass_kernel_spmd

    in_maps, with_ebias = _prep_inputs(x, w_comp, b_comp, w_enc, b_enc)
    nc = _get_program(with_ebias)
    res = run_bass_kernel_spmd(
        nc, in_maps, core_ids=list(range(NCORES)), trace=TRACE
    )
    LAST_RESULT = res
    out = np.empty((B, C, 2 * H, 2 * W), dtype=np.float32)
    for core in range(NCORES):
        b = core // 4
        h0 = (core % 4) * HSLICE
        o = np.asarray(res.results[core]["out"], dtype=np.float32)
        # rows n = (ro, w, r1, r2), cols = (g, b4, c)
        o = o.reshape(2, 16, 2, 2, 8, 4, 256)
        # -> (c, g, ro, r1, b4, w, r2)
        o = np.transpose(o, (6, 4, 0, 2, 5, 1, 3)).reshape(256, 32, 128)
        out[b, :, 2 * h0:2 * h0 + 32, :] = o
    return out
